# revision 7
# baseline (speedup 1.0000x reference)
"""AdmEdgeDetect Trainium2 kernel: 9x9 circular conv (8 filters) + grad-mag + threshold.

Data parallel across 8 NeuronCores: 2 images per core.
Conv strategy: for each output row-band (120 rows), the 9x9 conv is computed as
9 accumulating matmuls (one per horizontal tap dx). Each matmul's stationary
operand is a banded Toeplitz matrix W[f,dx][k,m] = filt[f, k-m, dx] mapping 128
input rows -> 120 output rows; the moving operand is the circularly padded row
band x[128, 1032] sliced at free-offset dx (horizontal shift is a free-dim
address offset). PSUM accumulates the 72 matmuls per filter bank.
Elementwise: q_s = fx^2+fy^2, m = max_s q_s, grads = sqrt(m),
t = exp(ln(base)*grads), w = ghi + (glo-ghi)*(t-1) with ghi=[t>1+u], glo=[t>=1+l].
"""
import sys

sys.path.insert(0, "/opt/trn_rl_repo")
sys.path.insert(0, "/opt/pypackages")

import math
import numpy as np

from concourse import bass, bacc, mybir
from concourse.bass_utils import run_bass_kernel_spmd
from concourse.tile import TileContext

H = W = 1024
K = 9
PAD = K // 2  # 4
NF = 8
BAND = 120            # output rows per band (input rows = 128)
NBANDS = 9            # 8 full bands of 120 + last band of 64
CHUNK = 512           # output cols per psum chunk
NCHUNK = W // CHUNK
IMGS_PER_CORE = 2
NCORES = 8

# dtype used for matmul operands (float32 = exact, float32r = fast ~tf32-ish)
MM_DT = mybir.dt.float32


def band_rows(i):
    """(row0, n_out_rows) for band i."""
    r0 = BAND * i
    m = min(BAND, H - r0)
    return r0, m


def build_toeplitz(filters):
    """[NF*K, 128, 120] stationary matrices: wt[f*9+dx][k, m] = filt[f, k-m, dx]."""
    filt = np.asarray(filters, dtype=np.float32).reshape(NF, K, K)
    wt = np.zeros((NF * K, 128, BAND), dtype=np.float32)
    for f in range(NF):
        for dx in range(K):
            mat = wt[f * K + dx]
            for dy in range(K):
                # input row k = m + dy  (band loads input rows r0-4 .. r0+123,
                # so local input row k corresponds to global r0 - PAD + k;
                # output local m is global r0 + m; tap dy = k - m)
                for m in range(BAND):
                    k = m + dy
                    if k < 128:
                        mat[k, m] = filt[f, dy, dx]
    # transpose to [128, NF*K, 120] so DMA partition dim is first
    return np.ascontiguousarray(wt.transpose(1, 0, 2))


def build_graph(base, u_thre, l_thre):
    lnb = float(math.log(float(base)))
    up1 = 1.0 + float(u_thre)
    lp1 = 1.0 + float(l_thre)

    nc = bacc.Bacc(None, target_bir_lowering=False)
    x_ext = nc.declare_dram_parameter(
        "x", [IMGS_PER_CORE, H + 2 * PAD, W + 2 * PAD], mybir.dt.float32,
        isOutput=False,
    )
    wt_ext = nc.declare_dram_parameter(
        "wt", [128, NF * K, BAND], mybir.dt.float32, isOutput=False
    )
    g_ext = nc.declare_dram_parameter(
        "g", [IMGS_PER_CORE, H, W], mybir.dt.float32, isOutput=True
    )
    w_ext = nc.declare_dram_parameter(
        "w", [IMGS_PER_CORE, H, W], mybir.dt.float32, isOutput=True
    )

    with TileContext(nc) as tc:
        with (
            tc.tile_pool(name="consts", bufs=1) as cpool,
            tc.tile_pool(name="xb", bufs=3) as xpool,
            tc.tile_pool(name="ps", bufs=1, space="PSUM") as pspool,
            tc.tile_pool(name="ew", bufs=2) as epool,
        ):
            wt_sb = cpool.tile([128, NF * K, BAND], MM_DT, tag="wt")
            nc.sync.dma_start(out=wt_sb[:, :, :], in_=wt_ext[:, :, :])

            for img in range(IMGS_PER_CORE):
                for band in range(NBANDS):
                    r0, mrows = band_rows(band)
                    xb = xpool.tile([128, W + 2 * PAD], MM_DT, tag="xb")
                    # padded row p maps to global row p - PAD, so band i's
                    # input rows 120i-4 .. 120i+123 are padded rows 120i..+127
                    navail = min(128, H + 2 * PAD - r0)
                    nc.sync.dma_start(
                        out=xb[0:navail, :], in_=x_ext[img, r0 : r0 + navail, :]
                    )

                    for ch in range(NCHUNK):
                        c0 = ch * CHUNK
                        ps = [
                            pspool.tile(
                                [128, CHUNK], mybir.dt.float32,
                                tag=f"ps{f}", name=f"ps{f}",
                            )
                            for f in range(NF)
                        ]
                        for f in range(NF):
                            for dx in range(K):
                                nc.tensor.matmul(
                                    ps[f][0:mrows, :],
                                    lhsT=wt_sb[0:navail, f * K + dx, 0:mrows],
                                    rhs=xb[0:navail, c0 + dx : c0 + dx + CHUNK],
                                    start=(dx == 0),
                                    stop=(dx == K - 1),
                                )
                        # elementwise: ps[2s]=fx_s, ps[2s+1]=fy_s
                        qs = []
                        for s in range(4):
                            sy = epool.tile([128, CHUNK], mybir.dt.float32, tag=f"sy{s}")
                            nc.scalar.square(sy[0:mrows, :], ps[2 * s + 1][0:mrows, :])
                            tx = epool.tile([128, CHUNK], mybir.dt.float32, tag=f"tx{s}")
                            nc.scalar.square(tx[0:mrows, :], ps[2 * s][0:mrows, :])
                            q = epool.tile([128, CHUNK], mybir.dt.float32, tag=f"q{s}")
                            nc.vector.tensor_add(
                                q[0:mrows, :], tx[0:mrows, :], sy[0:mrows, :]
                            )
                            qs.append(q)
                        m01 = epool.tile([128, CHUNK], mybir.dt.float32, tag="m01")
                        nc.vector.tensor_max(
                            m01[0:mrows, :], qs[0][0:mrows, :], qs[1][0:mrows, :]
                        )
                        m23 = epool.tile([128, CHUNK], mybir.dt.float32, tag="m23")
                        nc.vector.tensor_max(
                            m23[0:mrows, :], qs[2][0:mrows, :], qs[3][0:mrows, :]
                        )
                        mm = epool.tile([128, CHUNK], mybir.dt.float32, tag="mm")
                        nc.vector.tensor_max(
                            mm[0:mrows, :], m01[0:mrows, :], m23[0:mrows, :]
                        )
                        g = epool.tile([128, CHUNK], mybir.dt.float32, tag="g")
                        nc.scalar.sqrt(g[0:mrows, :], mm[0:mrows, :])
                        t = epool.tile([128, CHUNK], mybir.dt.float32, tag="t")
                        nc.scalar.activation(
                            t[0:mrows, :],
                            g[0:mrows, :],
                            mybir.ActivationFunctionType.Exp,
                            scale=lnb,
                        )
                        ghi = epool.tile([128, CHUNK], mybir.dt.float32, tag="ghi")
                        nc.vector.tensor_scalar(
                            ghi[0:mrows, :], t[0:mrows, :], up1, None,
                            mybir.AluOpType.is_gt,
                        )
                        glo = epool.tile([128, CHUNK], mybir.dt.float32, tag="glo")
                        nc.vector.tensor_scalar(
                            glo[0:mrows, :], t[0:mrows, :], lp1, None,
                            mybir.AluOpType.is_ge,
                        )
                        d = epool.tile([128, CHUNK], mybir.dt.float32, tag="d")
                        nc.vector.tensor_sub(
                            d[0:mrows, :], glo[0:mrows, :], ghi[0:mrows, :]
                        )
                        w0 = epool.tile([128, CHUNK], mybir.dt.float32, tag="w0")
                        nc.vector.tensor_scalar_add(w0[0:mrows, :], t[0:mrows, :], -1.0)
                        p = epool.tile([128, CHUNK], mybir.dt.float32, tag="p")
                        nc.vector.tensor_mul(
                            p[0:mrows, :], d[0:mrows, :], w0[0:mrows, :]
                        )
                        wv = epool.tile([128, CHUNK], mybir.dt.float32, tag="wv")
                        nc.vector.tensor_add(
                            wv[0:mrows, :], ghi[0:mrows, :], p[0:mrows, :]
                        )
                        nc.sync.dma_start(
                            out=g_ext[img, r0 : r0 + mrows, c0 : c0 + CHUNK],
                            in_=g[0:mrows, :],
                        )
                        nc.sync.dma_start(
                            out=w_ext[img, r0 : r0 + mrows, c0 : c0 + CHUNK],
                            in_=wv[0:mrows, :],
                        )
    nc.compile()
    return nc


def prepare(inputs):
    x = np.asarray(inputs["x"], dtype=np.float32).reshape(16, H, W)
    x = np.pad(x, ((0, 0), (PAD, PAD), (PAD, PAD)), mode="wrap")
    wt = build_toeplitz(inputs["filters"])
    nc = build_graph(
        float(inputs["base"]), float(inputs["u_thre"]), float(inputs["l_thre"])
    )
    in_maps = []
    for c in range(NCORES):
        in_maps.append(
            {
                "x": np.ascontiguousarray(x[c * IMGS_PER_CORE : (c + 1) * IMGS_PER_CORE]),
                "wt": wt,
            }
        )
    return in_maps, nc


def kernel(x, filters, base, u_thre, l_thre, idx, ite):
    in_maps, nc = prepare(
        {"x": x, "filters": filters, "base": base, "u_thre": u_thre, "l_thre": l_thre}
    )
    res = run_bass_kernel_spmd(nc, in_maps, core_ids=list(range(NCORES))).results
    g = np.concatenate([res[c]["g"] for c in range(NCORES)], axis=0)
    w = np.concatenate([res[c]["w"] for c in range(NCORES)], axis=0)
    return g.reshape(16, 1, H, W), w.reshape(16, 1, H, W)


# revision 8
# speedup vs baseline: 1.2360x; 1.2360x over previous
"""AdmEdgeDetect Trainium2 kernel: 9x9 circular conv (8 filters) + grad-mag + threshold.

Data parallel across 8 NeuronCores: 2 images per core.
Conv strategy: for each output row-band (120 rows), the 9x9 conv is computed as
9 accumulating matmuls (one per horizontal tap dx). Each matmul's stationary
operand is a banded Toeplitz matrix W[f,dx][k,m] = filt[f, k-m, dx] mapping 128
input rows -> 120 output rows; the moving operand is the circularly padded row
band x[128, 1032] sliced at free-offset dx (horizontal shift is a free-dim
address offset). PSUM accumulates the 72 matmuls per filter bank.
Elementwise: q_s = fx^2+fy^2, m = max_s q_s, grads = sqrt(m),
t = exp(ln(base)*grads), w = ghi + (glo-ghi)*(t-1) with ghi=[t>1+u], glo=[t>=1+l].
"""
import sys

sys.path.insert(0, "/opt/trn_rl_repo")
sys.path.insert(0, "/opt/pypackages")

import math
import numpy as np

from concourse import bass, bacc, mybir
from concourse.bass_utils import run_bass_kernel_spmd
from concourse.tile import TileContext

H = W = 1024
K = 9
PAD = K // 2  # 4
NF = 8
BAND = 120            # output rows per band (input rows = 128)
NBANDS = 9            # 8 full bands of 120 + last band of 64
CHUNK = 512           # output cols per psum chunk
NCHUNK = W // CHUNK
IMGS_PER_CORE = 2
NCORES = 8

# dtype used for matmul operands (float32 = exact, float32r = fast ~tf32-ish)
MM_DT = mybir.dt.float32
# split-bf16 conv: x=xh+xl, W=Wh+Wl; accumulate Wh@xh + Wl@xh + Wh@xl (bf16
# matmuls run 1 cycle/row vs 4 for fp32; combined error ~1e-6 relative)
MM_SPLIT = True


def band_rows(i):
    """(row0, n_out_rows) for band i."""
    r0 = BAND * i
    m = min(BAND, H - r0)
    return r0, m


def build_toeplitz(filters):
    """[NF*K, 128, 120] stationary matrices: wt[f*9+dx][k, m] = filt[f, k-m, dx]."""
    filt = np.asarray(filters, dtype=np.float32).reshape(NF, K, K)
    wt = np.zeros((NF * K, 128, BAND), dtype=np.float32)
    for f in range(NF):
        for dx in range(K):
            mat = wt[f * K + dx]
            for dy in range(K):
                # input row k = m + dy  (band loads input rows r0-4 .. r0+123,
                # so local input row k corresponds to global r0 - PAD + k;
                # output local m is global r0 + m; tap dy = k - m)
                for m in range(BAND):
                    k = m + dy
                    if k < 128:
                        mat[k, m] = filt[f, dy, dx]
    # transpose to [128, NF*K, 120] so DMA partition dim is first
    return np.ascontiguousarray(wt.transpose(1, 0, 2))


def build_graph(base, u_thre, l_thre):
    lnb = float(math.log(float(base)))
    up1 = 1.0 + float(u_thre)
    lp1 = 1.0 + float(l_thre)

    nc = bacc.Bacc(None, target_bir_lowering=False)
    x_ext = nc.declare_dram_parameter(
        "x", [IMGS_PER_CORE, H + 2 * PAD, W + 2 * PAD], mybir.dt.float32,
        isOutput=False,
    )
    if MM_SPLIT:
        wt_hi_ext = nc.declare_dram_parameter(
            "wt_hi", [128, NF * K, BAND], mybir.dt.bfloat16, isOutput=False
        )
        wt_lo_ext = nc.declare_dram_parameter(
            "wt_lo", [128, NF * K, BAND], mybir.dt.bfloat16, isOutput=False
        )
    else:
        wt_ext = nc.declare_dram_parameter(
            "wt", [128, NF * K, BAND], mybir.dt.float32, isOutput=False
        )
    g_ext = nc.declare_dram_parameter(
        "g", [IMGS_PER_CORE, H, W], mybir.dt.float32, isOutput=True
    )
    w_ext = nc.declare_dram_parameter(
        "w", [IMGS_PER_CORE, H, W], mybir.dt.float32, isOutput=True
    )

    with TileContext(nc) as tc:
        with (
            tc.tile_pool(name="consts", bufs=1) as cpool,
            tc.tile_pool(name="xb", bufs=3) as xpool,
            tc.tile_pool(name="ps", bufs=1, space="PSUM") as pspool,
            tc.tile_pool(name="ew", bufs=2) as epool,
        ):
            if MM_SPLIT:
                wt_hi_sb = cpool.tile(
                    [128, NF * K, BAND], mybir.dt.bfloat16, tag="wth"
                )
                wt_lo_sb = cpool.tile(
                    [128, NF * K, BAND], mybir.dt.bfloat16, tag="wtl"
                )
                nc.sync.dma_start(out=wt_hi_sb[:, :, :], in_=wt_hi_ext[:, :, :])
                nc.sync.dma_start(out=wt_lo_sb[:, :, :], in_=wt_lo_ext[:, :, :])
            else:
                wt_sb = cpool.tile([128, NF * K, BAND], MM_DT, tag="wt")
                nc.sync.dma_start(out=wt_sb[:, :, :], in_=wt_ext[:, :, :])

            for img in range(IMGS_PER_CORE):
                for band in range(NBANDS):
                    r0, mrows = band_rows(band)
                    xb = xpool.tile([128, W + 2 * PAD], MM_DT, tag="xb")
                    # padded row p maps to global row p - PAD, so band i's
                    # input rows 120i-4 .. 120i+123 are padded rows 120i..+127
                    navail = min(128, H + 2 * PAD - r0)
                    nc.sync.dma_start(
                        out=xb[0:navail, :], in_=x_ext[img, r0 : r0 + navail, :]
                    )
                    if MM_SPLIT:
                        xh = xpool.tile(
                            [128, W + 2 * PAD], mybir.dt.bfloat16, tag="xh"
                        )
                        xl = xpool.tile(
                            [128, W + 2 * PAD], mybir.dt.bfloat16, tag="xl"
                        )
                        nc.vector.tensor_copy(xh[0:navail, :], xb[0:navail, :])
                        nc.vector.tensor_sub(
                            xl[0:navail, :], xb[0:navail, :], xh[0:navail, :]
                        )

                    for ch in range(NCHUNK):
                        c0 = ch * CHUNK
                        ps = [
                            pspool.tile(
                                [128, CHUNK], mybir.dt.float32,
                                tag=f"ps{f}", name=f"ps{f}",
                            )
                            for f in range(NF)
                        ]
                        for f in range(NF):
                            if MM_SPLIT:
                                terms = []
                                for dx in range(K):
                                    i = f * K + dx
                                    terms += [
                                        (wt_hi_sb, xh, i, dx),
                                        (wt_lo_sb, xh, i, dx),
                                        (wt_hi_sb, xl, i, dx),
                                    ]
                                for t_i, (wsb, xsb, i, dx) in enumerate(terms):
                                    nc.tensor.matmul(
                                        ps[f][0:mrows, :],
                                        lhsT=wsb[0:navail, i, 0:mrows],
                                        rhs=xsb[0:navail, c0 + dx : c0 + dx + CHUNK],
                                        start=(t_i == 0),
                                        stop=(t_i == len(terms) - 1),
                                    )
                            else:
                                for dx in range(K):
                                    nc.tensor.matmul(
                                        ps[f][0:mrows, :],
                                        lhsT=wt_sb[0:navail, f * K + dx, 0:mrows],
                                        rhs=xb[0:navail, c0 + dx : c0 + dx + CHUNK],
                                        start=(dx == 0),
                                        stop=(dx == K - 1),
                                    )
                        # elementwise: ps[2s]=fx_s, ps[2s+1]=fy_s
                        qs = []
                        for s in range(4):
                            sy = epool.tile([128, CHUNK], mybir.dt.float32, tag=f"sy{s}")
                            nc.scalar.square(sy[0:mrows, :], ps[2 * s + 1][0:mrows, :])
                            tx = epool.tile([128, CHUNK], mybir.dt.float32, tag=f"tx{s}")
                            nc.scalar.square(tx[0:mrows, :], ps[2 * s][0:mrows, :])
                            q = epool.tile([128, CHUNK], mybir.dt.float32, tag=f"q{s}")
                            nc.vector.tensor_add(
                                q[0:mrows, :], tx[0:mrows, :], sy[0:mrows, :]
                            )
                            qs.append(q)
                        m01 = epool.tile([128, CHUNK], mybir.dt.float32, tag="m01")
                        nc.vector.tensor_max(
                            m01[0:mrows, :], qs[0][0:mrows, :], qs[1][0:mrows, :]
                        )
                        m23 = epool.tile([128, CHUNK], mybir.dt.float32, tag="m23")
                        nc.vector.tensor_max(
                            m23[0:mrows, :], qs[2][0:mrows, :], qs[3][0:mrows, :]
                        )
                        mm = epool.tile([128, CHUNK], mybir.dt.float32, tag="mm")
                        nc.vector.tensor_max(
                            mm[0:mrows, :], m01[0:mrows, :], m23[0:mrows, :]
                        )
                        g = epool.tile([128, CHUNK], mybir.dt.float32, tag="g")
                        nc.scalar.sqrt(g[0:mrows, :], mm[0:mrows, :])
                        t = epool.tile([128, CHUNK], mybir.dt.float32, tag="t")
                        nc.scalar.activation(
                            t[0:mrows, :],
                            g[0:mrows, :],
                            mybir.ActivationFunctionType.Exp,
                            scale=lnb,
                        )
                        ghi = epool.tile([128, CHUNK], mybir.dt.float32, tag="ghi")
                        nc.vector.tensor_scalar(
                            ghi[0:mrows, :], t[0:mrows, :], up1, None,
                            mybir.AluOpType.is_gt,
                        )
                        glo = epool.tile([128, CHUNK], mybir.dt.float32, tag="glo")
                        nc.vector.tensor_scalar(
                            glo[0:mrows, :], t[0:mrows, :], lp1, None,
                            mybir.AluOpType.is_ge,
                        )
                        d = epool.tile([128, CHUNK], mybir.dt.float32, tag="d")
                        nc.vector.tensor_sub(
                            d[0:mrows, :], glo[0:mrows, :], ghi[0:mrows, :]
                        )
                        w0 = epool.tile([128, CHUNK], mybir.dt.float32, tag="w0")
                        nc.vector.tensor_scalar_add(w0[0:mrows, :], t[0:mrows, :], -1.0)
                        p = epool.tile([128, CHUNK], mybir.dt.float32, tag="p")
                        nc.vector.tensor_mul(
                            p[0:mrows, :], d[0:mrows, :], w0[0:mrows, :]
                        )
                        wv = epool.tile([128, CHUNK], mybir.dt.float32, tag="wv")
                        nc.vector.tensor_add(
                            wv[0:mrows, :], ghi[0:mrows, :], p[0:mrows, :]
                        )
                        nc.sync.dma_start(
                            out=g_ext[img, r0 : r0 + mrows, c0 : c0 + CHUNK],
                            in_=g[0:mrows, :],
                        )
                        nc.sync.dma_start(
                            out=w_ext[img, r0 : r0 + mrows, c0 : c0 + CHUNK],
                            in_=wv[0:mrows, :],
                        )
    nc.compile()
    return nc


def prepare(inputs):
    x = np.asarray(inputs["x"], dtype=np.float32).reshape(16, H, W)
    x = np.pad(x, ((0, 0), (PAD, PAD), (PAD, PAD)), mode="wrap")
    wt = build_toeplitz(inputs["filters"])
    if MM_SPLIT:
        import ml_dtypes

        wt_hi = wt.astype(ml_dtypes.bfloat16)
        wt_lo = (wt - wt_hi.astype(np.float32)).astype(ml_dtypes.bfloat16)
    nc = build_graph(
        float(inputs["base"]), float(inputs["u_thre"]), float(inputs["l_thre"])
    )
    in_maps = []
    for c in range(NCORES):
        m = {"x": np.ascontiguousarray(x[c * IMGS_PER_CORE : (c + 1) * IMGS_PER_CORE])}
        if MM_SPLIT:
            m["wt_hi"] = wt_hi
            m["wt_lo"] = wt_lo
        else:
            m["wt"] = wt
        in_maps.append(m)
    return in_maps, nc


def kernel(x, filters, base, u_thre, l_thre, idx, ite):
    in_maps, nc = prepare(
        {"x": x, "filters": filters, "base": base, "u_thre": u_thre, "l_thre": l_thre}
    )
    res = run_bass_kernel_spmd(nc, in_maps, core_ids=list(range(NCORES))).results
    g = np.concatenate([res[c]["g"] for c in range(NCORES)], axis=0)
    w = np.concatenate([res[c]["w"] for c in range(NCORES)], axis=0)
    return g.reshape(16, 1, H, W), w.reshape(16, 1, H, W)
